# revision 8
# baseline (speedup 1.0000x reference)
"""KernelConv2D (per-pixel dynamic 5x5 depthwise conv) on 8 TRN2 NeuronCores.

Problem: out[b,c,h,w] = sum_{i,j} x_edgepad[b,c,h+i,w+j] * K[b,c,i,j,h,w]
with input [4,32,128,128] f32 and kernel [4,800,128,128] f32 (800 = 32*25).

Sharding: every (b,c) plane is independent, so flatten to 128 planes and put
the plane index on the SBUF partition axis. Each core takes 16 output ROWS of
all 128 planes (row-sharding). With (h, w) both living in the free dimension,
both conv shifts are constant free-dim offsets -> the 5x5 taps of the input
window are expressed as a single overlapping access pattern, no halo exchange
or partition-shifted copies on device. Host pre-pads the input with edge
replication and slices per-core row bands (incl. 2-row halo).

Per core HBM traffic: K 26.2MB + X 1.35MB + out 1.05MB ~= 28.6MB -> ~80us at
~358 GB/s/core: the memory roofline for this problem. Compute is split so DVE
(products + 9-segment reduce) and GpSimd (16-segment add tree) both stay at or
under the DMA time.
"""

import sys

import numpy as np

sys.path.insert(0, "/opt/trn_rl_repo")

import concourse.bacc as bacc
import concourse.bass as bass
import concourse.tile as tile
from concourse import mybir
from concourse.ap import AP
from concourse.bass_utils import run_bass_kernel_spmd

N_CORES = 8
B, C, H, W, KS = 4, 32, 128, 128, 5
NPLANES = B * C          # 128 -> partition axis
NTAPS = KS * KS          # 25
ROWS_PER_CORE = H // N_CORES   # 16
ROWS_PER_CHUNK = 2
NCHUNK = ROWS_PER_CORE // ROWS_PER_CHUNK   # 8
FDW = ROWS_PER_CHUNK * W                   # 256 output elems per chunk-partition
XW = W + KS - 1                            # 132 padded row width
XROWS = ROWS_PER_CORE + KS - 1             # 20 rows incl halo
F32 = mybir.dt.float32

# Reduction split: DVE tensor_reduce sums taps [0, DVE_SEGS); GpSimd sums the
# remaining 16 taps with a 4-level pairwise tree.
DVE_SEGS = 9
GP_SEGS = NTAPS - DVE_SEGS  # 16

_compiled = None


def _build_program():
    nc = bacc.Bacc(
        "TRN2",
        target_bir_lowering=False,
        debug=False,
        enable_asserts=False,
        num_devices=N_CORES,
    )
    # Host pre-arranges k as [plane][chunk][tap][h2][w] so each chunk load is
    # one contiguous per-partition run (few DMA descriptors, near line rate).
    xd = nc.declare_dram_parameter("x", [NPLANES, XROWS * XW], F32, isOutput=False)
    kd = nc.declare_dram_parameter(
        "k", [NPLANES, NCHUNK * NTAPS * FDW], F32, isOutput=False
    )
    od = nc.declare_dram_parameter("o", [NPLANES, NCHUNK * FDW], F32, isOutput=True)

    with tile.TileContext(nc) as tc:
        with (
            tc.tile_pool(name="xpool", bufs=1) as xpool,
            tc.tile_pool(name="kpool", bufs=2) as kpool,
            tc.tile_pool(name="ppool", bufs=2) as ppool,
            tc.tile_pool(name="gpool", bufs=2) as gpool,
            tc.tile_pool(name="rpool", bufs=2) as rpool,
            tc.tile_pool(name="opool", bufs=3) as opool,
        ):
            # Whole padded input band for this core, resident for the kernel.
            xt = xpool.tile([NPLANES, XROWS * XW], F32)
            nc.sync.dma_start(out=xt[:], in_=xd.ap())
            xt_ap = xt[:]
            xt_pdim = xt_ap.ap[0]  # (partition step, 128)

            for ch in range(NCHUNK):
                h0 = ch * ROWS_PER_CHUNK
                kt = kpool.tile([NPLANES, NTAPS * FDW], F32, tag="kt")
                nc.sync.dma_start(
                    out=kt[:],
                    in_=kd.ap()[:, ch * NTAPS * FDW : (ch + 1) * NTAPS * FDW],
                )
                pt = ppool.tile([NPLANES, NTAPS * FDW], F32, tag="pt")
                # Products: one op per vertical tap i covers the 5 horizontal
                # taps j as an overlapping strided window of the X band.
                for i in range(KS):
                    k_view = kt[:, i * KS * FDW : (i + 1) * KS * FDW].rearrange(
                        "p (j h w) -> p j h w", j=KS, h=ROWS_PER_CHUNK, w=W
                    )
                    p_view = pt[:, i * KS * FDW : (i + 1) * KS * FDW].rearrange(
                        "p (j h w) -> p j h w", j=KS, h=ROWS_PER_CHUNK, w=W
                    )
                    x_view = AP(
                        xt_ap.tensor,
                        xt_ap.offset + (h0 + i) * XW,
                        [xt_pdim, (1, KS), (XW, ROWS_PER_CHUNK), (1, W)],
                    )
                    nc.vector.tensor_mul(p_view, k_view, x_view)

                # GpSimd: pairwise add tree over taps [DVE_SEGS, 25) -> 1 seg.
                gt = gpool.tile([NPLANES, 15 * FDW], F32, tag="gt")
                nc.gpsimd.tensor_add(
                    gt[:, 0 : 8 * FDW],
                    pt[:, DVE_SEGS * FDW : (DVE_SEGS + 8) * FDW],
                    pt[:, (DVE_SEGS + 8) * FDW : NTAPS * FDW],
                )
                nc.gpsimd.tensor_add(
                    gt[:, 8 * FDW : 12 * FDW], gt[:, 0 : 4 * FDW], gt[:, 4 * FDW : 8 * FDW]
                )
                nc.gpsimd.tensor_add(
                    gt[:, 12 * FDW : 14 * FDW],
                    gt[:, 8 * FDW : 10 * FDW],
                    gt[:, 10 * FDW : 12 * FDW],
                )
                nc.gpsimd.tensor_add(
                    gt[:, 14 * FDW : 15 * FDW],
                    gt[:, 12 * FDW : 13 * FDW],
                    gt[:, 13 * FDW : 14 * FDW],
                )

                # DVE: segmented reduce of taps [0, DVE_SEGS) via a transposed
                # view (innermost axis = tap).
                rt = rpool.tile([NPLANES, FDW], F32, tag="rt")
                pt_ap = pt[:]
                red_in = AP(
                    pt_ap.tensor,
                    pt_ap.offset,
                    [pt_ap.ap[0], (1, FDW), (FDW, DVE_SEGS)],
                )
                nc.vector.tensor_reduce(
                    rt[:], red_in, axis=mybir.AxisListType.X, op=mybir.AluOpType.add
                )

                ot = opool.tile([NPLANES, FDW], F32, tag="ot")
                nc.vector.tensor_add(ot[:], rt[:], gt[:, 14 * FDW : 15 * FDW])
                nc.sync.dma_start(
                    out=od.ap()[:, ch * FDW : (ch + 1) * FDW], in_=ot[:]
                )

    nc.compile()
    return nc


def _get_program():
    global _compiled
    if _compiled is None:
        _compiled = _build_program()
    return _compiled


def _shard_inputs(input: np.ndarray, kernel: np.ndarray):
    x = np.ascontiguousarray(input, dtype=np.float32).reshape(NPLANES, H, W)
    xp = np.pad(x, ((0, 0), (2, 2), (2, 2)), mode="edge")  # [128, 132, 132]
    k = np.ascontiguousarray(kernel, dtype=np.float32).reshape(
        NPLANES, NTAPS, H, W
    )
    in_maps = []
    for c in range(N_CORES):
        r0 = c * ROWS_PER_CORE
        # [plane][tap][16 rows][w] -> [plane][chunk][tap][2 rows][w], flattened
        # per plane so each chunk is one contiguous run.
        kc = k[:, :, r0 : r0 + ROWS_PER_CORE, :].reshape(
            NPLANES, NTAPS, NCHUNK, ROWS_PER_CHUNK, W
        )
        kc = np.ascontiguousarray(kc.transpose(0, 2, 1, 3, 4)).reshape(
            NPLANES, NCHUNK * NTAPS * FDW
        )
        in_maps.append(
            {
                "x": np.ascontiguousarray(
                    xp[:, r0 : r0 + XROWS, :]
                ).reshape(NPLANES, XROWS * XW),
                "k": kc,
            }
        )
    return in_maps


last_results = None  # BassKernelResults of the most recent run (for profiling)


def kernel(input: np.ndarray, kernel: np.ndarray, _trace: bool = False):
    global last_results
    nc = _get_program()
    in_maps = _shard_inputs(input, kernel)
    res = run_bass_kernel_spmd(nc, in_maps, list(range(N_CORES)), trace=_trace)
    last_results = res
    out = np.empty((NPLANES, H, W), dtype=np.float32)
    for c in range(N_CORES):
        out[:, c * ROWS_PER_CORE : (c + 1) * ROWS_PER_CORE, :] = res.results[c][
            "o"
        ].reshape(NPLANES, ROWS_PER_CORE, W)
    return out.reshape(B, C, H, W)


if __name__ == "__main__":
    rng = np.random.default_rng(0)
    inp = rng.standard_normal((B, C, H, W), dtype=np.float32)
    kern = rng.standard_normal((B, C * NTAPS, H, W), dtype=np.float32)
    out = kernel(inp, kern)
    print("ran ok", out.shape, out.dtype)
